# revision 2
# baseline (speedup 1.0000x reference)
"""Affinity module (L2-normalize channels -> gram -> L1 row-normalize) on 8 TRN2 cores.

Math: with y = x / ||x_col||_2 (per spatial column), the reference output is
    out[i, j] = sim[i, j] / sum_j' |sim[i, j']|,   sim = y^T y.
Any scaling of row i cancels in the L1 row normalization, so computing
    u[i, j] = (y^T y)[i, j]  for a slab of rows i, then u / rowsum(|u|)
matches the reference exactly (up to fp rounding).

Sharding: 8 cores = 2 batches x 4 row-slabs of 2304. Each core receives its
batch's x[C, N] with columns ROTATED so that its slab is always columns
0:2304 -> identical IR on every core (one SPMD NEFF); the host un-rotates the
output columns afterwards. lhsT slices come straight from the resident
normalized y tiles.

Compute: fp32r matmuls (measured: full bf16-rate on TRN2, ~1.5e-4 rel err).
"""
import os

import numpy as np

import concourse.bass as bass
import concourse.tile as tile
from concourse import bacc, bass_isa, mybir
from concourse.bass_utils import run_bass_kernel_spmd

B, C, H, W = 2, 512, 96, 96
N = H * W                  # 9216
NCORES = 8
SLABS = 4                  # row-slabs per batch
SLAB = N // SLABS          # 2304
NT = 512                   # free-dim tile (one PSUM bank of fp32)
NCH = N // NT              # 18 column chunks
KT = C // 128              # 4 contraction sub-tiles
MB = SLAB // 128           # 18 m-blocks per core
NGRP = 6                   # PSUM banks per matmul group

f32 = mybir.dt.float32
f32r = mybir.dt.float32r
f16 = mybir.dt.float16


def _build():
    nc = bacc.Bacc(trn_type="TRN2", num_devices=NCORES)
    x = nc.dram_tensor("x", [C, N], f32, kind="ExternalInput")
    out = nc.dram_tensor("out", [SLAB, N], f32, kind="ExternalOutput")

    with tile.TileContext(nc) as tc:
        with (
            tc.tile_pool(name="y", bufs=1) as py,
            tc.tile_pool(name="ld", bufs=5) as pld,
            tc.tile_pool(name="nrm1", bufs=1) as pnrm1,
            tc.tile_pool(name="nrm2", bufs=2) as pnrm2,
            tc.tile_pool(name="t", bufs=20) as pt,
            tc.tile_pool(name="u", bufs=5) as pu,
            tc.tile_pool(name="rs", bufs=2) as prs,
            tc.tile_pool(name="ps", bufs=8, space="PSUM") as pps,
        ):
            # ---- prologue: per column chunk, L2-normalize into f32r y tiles
            ytiles = [[None] * NCH for _ in range(KT)]
            for c in range(NCH):
                xch = []
                for k in range(KT):
                    t_ld = pld.tile([128, NT], f32, tag="ld", name=f"ld{c}_{k}")
                    nc.sync.dma_start(
                        t_ld[:], x[k * 128:(k + 1) * 128, c * NT:(c + 1) * NT]
                    )
                    xch.append(t_ld)
                # sum of squares over channels: ACT squares, DVE add tree
                sq0 = pnrm1.tile([128, NT], f32, tag="sqa", name=f"sqa{c}")
                sq1 = pnrm1.tile([128, NT], f32, tag="sqb", name=f"sqb{c}")
                acc0 = pnrm1.tile([128, NT], f32, tag="acca", name=f"acca{c}")
                acc1 = pnrm1.tile([128, NT], f32, tag="accb", name=f"accb{c}")
                ssum = pnrm1.tile([128, NT], f32, tag="ssum", name=f"ssum{c}")
                nc.scalar.square(sq0[:], xch[0][:])
                nc.scalar.square(sq1[:], xch[1][:])
                nc.vector.tensor_add(acc0[:], sq0[:], sq1[:])
                nc.scalar.square(sq0[:], xch[2][:])
                nc.scalar.square(sq1[:], xch[3][:])
                nc.vector.tensor_add(acc1[:], sq0[:], sq1[:])
                nc.vector.tensor_add(ssum[:], acc0[:], acc1[:])
                # cross-partition column sum, broadcast to all partitions
                parb = pnrm2.tile([128, NT], f32, tag="parb", name=f"parb{c}")
                nc.gpsimd.partition_all_reduce(
                    parb[:], ssum[:], channels=128, reduce_op=bass_isa.ReduceOp.add
                )
                l2b = pnrm1.tile([128, NT], f32, tag="l2b", name=f"l2b{c}")
                nc.scalar.sqrt(l2b[:], parb[:])
                rb = pnrm2.tile([128, NT], f32, tag="rb", name=f"rb{c}")
                nc.vector.reciprocal(rb[:], l2b[:])
                for k in range(KT):
                    ty = py.tile([128, NT], f32r, tag=f"y{k}_{c}", name=f"y{k}_{c}")
                    nc.vector.tensor_mul(ty[:], xch[k][:], rb[:])
                    ytiles[k][c] = ty

            # ---- main: u = y_slab^T @ y, L1 row-normalize, store
            for m in range(MB):
                rs_parts = prs.tile([128, NCH], f32, tag="rsp", name=f"rsp{m}")
                ts = []
                for g in range(NCH // NGRP):
                    pss = []
                    for j in range(NGRP):
                        psj = pps.tile([128, NT], f32, tag="ps", name=f"ps{m}_{g}_{j}")
                        pss.append(psj)
                    for k in range(KT):
                        lhsT = ytiles[k][m // 4][:, (m % 4) * 128:(m % 4 + 1) * 128]
                        for j in range(NGRP):
                            nc.tensor.matmul(
                                pss[j][:],
                                lhsT,
                                ytiles[k][g * NGRP + j][:],
                                start=(k == 0),
                                stop=(k == KT - 1),
                            )
                    for j in range(NGRP):
                        n = g * NGRP + j
                        tt = pt.tile([128, NT], f16, tag="t", name=f"t{m}_{n}")
                        nc.vector.tensor_copy(tt[:], pss[j][:])
                        nc.vector.tensor_reduce(
                            rs_parts[:, n:n + 1],
                            pss[j][:],
                            axis=mybir.AxisListType.X,
                            op=mybir.AluOpType.add,
                            apply_absolute_value=True,
                        )
                        ts.append(tt)
                rs_tot = prs.tile([128, 1], f32, tag="rst", name=f"rst{m}")
                nc.vector.reduce_sum(rs_tot[:], rs_parts[:], axis=mybir.AxisListType.X)
                rinv = prs.tile([128, 1], f32, tag="rinv", name=f"rinv{m}")
                nc.vector.reciprocal(rinv[:], rs_tot[:])
                for n, tt in enumerate(ts):
                    ut = pu.tile([128, NT], f32, tag="u", name=f"u{m}_{n}")
                    nc.scalar.mul(ut[:], tt[:], rinv[:])
                    nc.sync.dma_start(
                        out[m * 128:(m + 1) * 128, n * NT:(n + 1) * NT], ut[:]
                    )

    nc.finalize()
    return nc


_NC = None


def _get_nc():
    global _NC
    if _NC is None:
        _NC = _build()
    return _NC


def kernel(x: np.ndarray) -> np.ndarray:
    x = np.ascontiguousarray(np.asarray(x), dtype=np.float32)
    assert x.shape == (B, C, H, W), x.shape
    xf = x.reshape(B, C, N)
    in_maps = []
    for core in range(NCORES):
        b, s = divmod(core, SLABS)
        in_maps.append({"x": np.ascontiguousarray(np.roll(xf[b], -s * SLAB, axis=1))})

    nc = _get_nc()
    res = run_bass_kernel_spmd(
        nc,
        in_maps,
        core_ids=list(range(NCORES)),
        trace=bool(os.environ.get("AFF_TRACE")),
    )
    if os.environ.get("AFF_TRACE"):
        kernel.last_exec_time_ns = res.exec_time_ns

    outp = np.empty((B, N, N), np.float32)
    for core in range(NCORES):
        b, s = divmod(core, SLABS)
        outp[b, s * SLAB:(s + 1) * SLAB, :] = np.roll(
            res.results[core]["out"], s * SLAB, axis=1
        )
    return outp
